# revision 1
# baseline (speedup 1.0000x reference)
"""Distributed Bass kernel for nn_AttentionLayer_88545045774526.

Causal attention layer: B=2, N=2048, D=1024, H=16 heads of HD=64.
Sharding: tensor-parallel over heads -- each of the 8 cores owns 2 heads
(both batches, all tokens). After attention, one 8-core AllToAll
redistributes attention outputs from head-sharded to token-sharded, and
each core runs the full output projection for its 512-token slice.

Matmul dtype: bf16 (fp32 streams at 1/4 PE rate); softmax in fp32.
Scores are computed transposed (S^T[k, q]) so that PV needs no on-chip
transpose of P; the softmax denominator rides a ones-column appended to V.
"""

import os
import sys

sys.path.insert(0, "/opt/trn_rl_repo")

import numpy as np
import ml_dtypes

import concourse.bass as bass
import concourse.mybir as mybir
import concourse.tile as tile
from concourse import bacc
from concourse import bass_utils
from concourse.masks import make_identity

BF16 = mybir.dt.bfloat16
F32 = mybir.dt.float32
NPBF16 = ml_dtypes.bfloat16

B, N, D = 2, 2048, 1024
H, HD = 16, 64
NC = 8                 # cores
HPC = 2                # heads per core
NT = B * N             # 4096 flat tokens (batch-major)
TB = 512               # token block (matmul moving dim)
P = 128

_BUILD_CACHE = {}
LAST_RESULT = None     # BassKernelResults of the most recent run (for test.py)


def _build_module(sim_mode=False, amplify=1, no_cc=False, probe="none"):
    """Build + compile the SPMD Bass graph (identical on all 8 cores).

    sim_mode=True replaces the AllToAll with a local DMA so the single-core
    TimelineSim cost model can run (it does not support collectives).
    amplify=N repeats the whole compute body N times (timing amplification).
    no_cc=True swaps the AllToAll for a local DMA (timing probe only).
    """
    key = (("nc_sim" if sim_mode else "nc") + (f"_x{amplify}" if amplify > 1 else "")
           + ("_nocc" if no_cc else "") + (f"_{probe}" if probe != "none" else ""))
    if key in _BUILD_CACHE:
        return _BUILD_CACHE[key]

    nc = bacc.Bacc(
        "TRN2",
        target_bir_lowering=False,
        debug=False,
        enable_asserts=False,
        num_devices=1 if sim_mode else NC,
    )

    # ---- I/O ----
    xt = nc.dram_tensor("xt", [D, NT], BF16, kind="ExternalInput")        # X^T, replicated
    wqkv = nc.dram_tensor("wqkv", [D, 3 * P], BF16, kind="ExternalInput")  # [Q|K|V] cols, 2 heads
    bqkv = nc.dram_tensor("bqkv", [3 * P], F32, kind="ExternalInput")
    wp = nc.dram_tensor("wp", [D, D], BF16, kind="ExternalInput")          # full W_proj
    bp = nc.dram_tensor("bp", [D], F32, kind="ExternalInput")
    tri = nc.dram_tensor("tri", [P, P], BF16, kind="ExternalInput")        # tri[p,f]=1 if f>=p
    out = nc.dram_tensor("out", [TB, D], F32, kind="ExternalOutput")       # Y for my 512 tokens

    # collective bounce buffers (internal DRAM)
    cc_in = nc.dram_tensor("cc_in", [NC * P, TB], BF16, kind="Internal")
    cc_out = nc.dram_tensor("cc_out", [NC * P, TB], BF16, kind="Internal")

    NTB = NT // TB          # 8 token blocks
    DC = D // P             # 8 contraction chunks
    KBB = N // P            # 16 k-blocks per batch
    QB = N // TB            # 4 q-blocks per batch

    with tile.TileContext(nc) as tc:
        with (
            tc.tile_pool(name="consts", bufs=1) as consts,
            tc.tile_pool(name="xt_pool", bufs=1) as xt_pool,
            tc.tile_pool(name="pers", bufs=1) as pers,
            tc.tile_pool(name="mm_psum", bufs=2, space="PSUM") as mm_psum,
            tc.tile_pool(name="pss_psum", bufs=2, space="PSUM") as pss_psum,
            tc.tile_pool(name="pso_psum", bufs=1, space="PSUM") as pso_psum,
            tc.tile_pool(name="work", bufs=4) as work,
            tc.tile_pool(name="small", bufs=3) as small,
            tc.tile_pool(name="dramsc", bufs=3, space="DRAM") as dramsc,
        ):
            # ---- constants / weights to SBUF ----
            bq_sb = consts.tile([P, 3], F32)
            nc.sync.dma_start(bq_sb, bqkv[:].rearrange("(g p) -> p g", p=P))
            wqkv_sb = consts.tile([P, DC, 3 * P], BF16)
            for dc in range(DC):
                nc.sync.dma_start(
                    wqkv_sb[:, dc],
                    wqkv[dc * P:(dc + 1) * P, :].rearrange("(c p) n -> p (c n)", p=P))
            tri_sb = consts.tile([P, P], BF16)
            nc.sync.dma_start(tri_sb, tri[:, :])
            ident = consts.tile([P, P], BF16)
            make_identity(nc, ident)
            # preload the exp table set so the ~2.7us ACT_TABLE_LOAD hides here
            ones1 = consts.tile([1, HD], F32)
            nc.vector.memset(ones1, 1.0)
            actwarm = consts.tile([1, 1], F32)
            nc.scalar.activation(actwarm, bq_sb[0:1, 0:1],
                                 mybir.ActivationFunctionType.Exp)

            # ---- X^T to SBUF (token-chunked so compute can start early) ----
            xt_sb = xt_pool.tile([P, DC, NT], BF16)
            for tb in range(NTB):
                if tb == 0:   # finer chunks so the first matmuls start sooner
                    for dc in range(DC):
                        nc.sync.dma_start(
                            xt_sb[:, dc, 0:TB],
                            xt[dc * P:(dc + 1) * P, 0:TB].rearrange(
                                "(c p) n -> p (c n)", p=P))
                else:
                    nc.sync.dma_start(
                        xt_sb[:, :, tb * TB:(tb + 1) * TB],
                        xt[:, tb * TB:(tb + 1) * TB].rearrange("(c p) n -> p c n", p=P),
                    )
            # needed only for the tail phases -- load after the hot-path DMAs
            bp_bc = consts.tile([P, D], F32)
            nc.sync.dma_start(bp_bc, bp[None, :].to_broadcast((P, D)))
            wp_sb = consts.tile([P, DC, D], BF16)
            nc.sync.dma_start(wp_sb, wp[:, :].rearrange("(c p) n -> p c n", p=P))

            # ---- QKV projection (transposed): qt/kt/vt = W^T X^T + b ----
            qt = pers.tile([P, NT], BF16)   # partitions: head h at rows h*64..h*64+64
            kt = pers.tile([P, NT], BF16)
            vt = pers.tile([P, NT], BF16)
            # vstore[:, kbg, h*65 : h*65+64] = V rows for token block kbg, head h
            # vstore[:, kbg, h*65+64] = 1.0  (softmax denominator trick)
            vstore = pers.tile([P, NT // P, 2 * (HD + 1)], BF16)
            nc.vector.memset(vstore[:, :, HD:HD + 1], 1.0)
            nc.vector.memset(vstore[:, :, 2 * HD + 1:2 * HD + 2], 1.0)
            dst = {0: qt, 1: kt, 2: vt}

            def emit_qkv_pair(tb0, rep=0):
                # two token blocks at once: each stationary weight tile is
                # loaded once and streams both blocks back-to-back
                tbs = (tb0, tb0 + 1)
                for cg in range(3):
                    pss = [mm_psum.tile([P, TB], F32, tag="mm_ps",
                                        name=f"ps{i}") for i in range(2)]
                    for dc in range(DC):
                        for i, tb in enumerate(tbs):
                            nc.tensor.matmul(
                                pss[i],
                                lhsT=wqkv_sb[:, dc, cg * P:(cg + 1) * P],
                                rhs=xt_sb[:, dc, tb * TB:(tb + 1) * TB],
                                start=(dc == 0),
                                stop=(dc == DC - 1),
                            )
                    for i, tb in enumerate(tbs):
                        nc.vector.tensor_add(
                            out=dst[cg][:, tb * TB:(tb + 1) * TB], in0=pss[i],
                            in1=bq_sb[:, cg:cg + 1].to_broadcast((P, TB)),
                        )
                # V^T -> V natural (PE transpose) for these token blocks
                for kbg in range(4 * tb0, 4 * (tb0 + 2)):
                    pst = mm_psum.tile([P, P], BF16, tag="mm_ps", name="pst")
                    nc.tensor.transpose(pst, vt[:, kbg * P:(kbg + 1) * P], ident)
                    nc.vector.tensor_copy(out=vstore[:, kbg, 0:HD], in_=pst[:, 0:HD])
                    nc.vector.tensor_copy(
                        out=vstore[:, kbg, HD + 1:2 * HD + 1], in_=pst[:, HD:2 * HD])

            def emit_attn(b, qb, rep=0):
                # S^T = (K^T block)^T-contraction @ Q^T, exp, PV
                qoff = b * N + qb * TB
                nkb = 4 * (qb + 1)          # causal: k blocks 0..4qb+3 (even)
                psO = [pso_psum.tile([HD + 1, TB], F32, tag=f"psO{h}",
                                     name=f"psO{h}")
                       for h in range(HPC)]
                for kp in range(nkb // 2):
                    pair = (2 * kp, 2 * kp + 1)
                    css = [max(kb - 4 * qb, 0) * P for kb in pair]
                    # two k-blocks share one 2-bank PSUM tile per head so a
                    # single exp covers both (halves ACT per-op overhead);
                    # S matmuls ordered h0,h1-adjacent so the K=64 pairs pack
                    # into disjoint PE row groups (tile_position 0/64).
                    psS = [pss_psum.tile([P, 2 * TB], F32, tag="psS",
                                         name=f"psS{h}") for h in range(HPC)]
                    for i, kb in enumerate(pair):
                        koff = b * N + kb * P
                        for h in range(HPC):
                            hp = h * HD
                            nc.tensor.matmul(
                                psS[h][:, i * TB + css[i]:(i + 1) * TB],
                                lhsT=kt[hp:hp + HD, koff:koff + P],
                                rhs=qt[hp:hp + HD, qoff + css[i]:qoff + TB],
                                start=True, stop=True,
                            )
                    pts = []
                    for h in range(HPC):
                        pt = work.tile([P, 2 * TB], BF16, tag=f"pt{h}",
                                       name="pt")
                        nc.scalar.activation(
                            pt[:, css[0]:2 * TB], psS[h][:, css[0]:2 * TB],
                            mybir.ActivationFunctionType.Exp,
                            scale=float(HD) ** -0.5,
                        )
                        pts.append(pt)
                    for h in range(HPC):
                        for i, kb in enumerate(pair):
                            jj = kb - 4 * qb
                            if jj >= 0:
                                # triangular mask on the diagonal square
                                cc = i * TB + css[i]
                                nc.vector.tensor_mul(
                                    out=pts[h][:, cc:cc + P],
                                    in0=pts[h][:, cc:cc + P], in1=tri_sb)
                            nc.tensor.matmul(
                                psO[h][:, css[i]:TB],
                                lhsT=vstore[:, b * KBB + kb,
                                            h * (HD + 1):(h + 1) * (HD + 1)],
                                rhs=pts[h][:, i * TB + css[i]:(i + 1) * TB],
                                start=(kb == 0), stop=(kb == nkb - 1),
                            )
                # evacuate PSUM fast, then normalize by the ones-row sum
                for h in range(HPC):
                    osb = small.tile([HD + 1, TB], F32, tag="osb", name="osb")
                    nc.vector.tensor_copy(out=osb, in_=psO[h])
                    rec = small.tile([1, TB], F32, tag="rec", name="rec")
                    nc.vector.reciprocal(rec, osb[HD:HD + 1, :])
                    # broadcast rec across 64 partitions via a DRAM bounce
                    # (DRAM source APs may broadcast) -- avoids GpSimd dispatch
                    recd = dramsc.tile([1, TB], F32, tag="recd", name="recd")
                    nc.sync.dma_start(recd, rec)
                    recb = small.tile([HD, TB], F32, tag="recb", name="recb")
                    nc.sync.dma_start(recb, recd[0:1, :].to_broadcast((HD, TB)))
                    ot = small.tile([HD, TB], BF16, tag="ot", name="ot")
                    nc.vector.tensor_mul(out=ot, in0=osb[0:HD, :], in1=recb)
                    shard = b * QB + qb
                    nc.sync.dma_start(
                        cc_in[shard * P + h * HD: shard * P + (h + 1) * HD, :],
                        ot)

            # interleave: attn(b, qb) depends exactly on qkv blocks b*4..b*4+qb
            otfull = pers.tile([P, DC, TB], BF16)
            ysb = pers.tile([P, TB // P, D], F32)
            for rep in range(amplify):
                do_qkv = probe != "no_qkv" or rep == 0
                do_attn = probe != "no_attn" or rep == 0
                do_tail = probe != "no_tail" or rep == 0
                for tp in range(NTB // 2):
                    if do_qkv:
                        emit_qkv_pair(2 * tp, rep)
                    if do_attn:
                        for t in (2 * tp, 2 * tp + 1):
                            emit_attn(t // 4, t % 4, rep)
                if not do_tail:
                    continue

                # ---- AllToAll: head-sharded -> token-sharded ----
                if sim_mode or no_cc:
                    nc.sync.dma_start(cc_out[:, :], cc_in[:, :])
                else:
                    nc.gpsimd.collective_compute(
                        "AllToAll",
                        mybir.AluOpType.bypass,
                        replica_groups=[list(range(NC))],
                        ins=[cc_in[:, :]],
                        outs=[cc_out[:, :]],
                    )

                # ---- output projection: Y = O^T^T W_p + b,  [512, 1024] ----
                nc.sync.dma_start(
                    otfull, cc_out[:, :].rearrange("(c p) n -> p c n", p=P))
                for tk in range(TB // P):
                    pss = [mm_psum.tile([P, TB], F32, tag="mm_ps",
                                        name=f"yp{i}") for i in range(2)]
                    for hc in range(DC):
                        for i in range(2):
                            nc.tensor.matmul(
                                pss[i],
                                lhsT=otfull[:, hc, tk * P:(tk + 1) * P],
                                rhs=wp_sb[:, hc, i * TB:(i + 1) * TB],
                                start=(hc == 0), stop=(hc == DC - 1),
                            )
                    for i in range(2):
                        nc.vector.tensor_add(
                            out=ysb[:, tk, i * TB:(i + 1) * TB], in0=pss[i],
                            in1=bp_bc[:, i * TB:(i + 1) * TB],
                        )
                    nc.sync.dma_start(out[tk * P:(tk + 1) * P, :],
                                      ysb[:, tk, :])

    nc.compile()
    _BUILD_CACHE[key] = nc
    return nc


def _make_in_maps(hidden_states, W_attn, b_attn, W_proj, b_proj):
    x = np.asarray(hidden_states, dtype=np.float32).reshape(NT, D)
    xt = np.ascontiguousarray(x.T).astype(NPBF16)
    wp = np.asarray(W_proj, dtype=np.float32).astype(NPBF16)
    bp = np.asarray(b_proj, dtype=np.float32)
    W = np.asarray(W_attn, dtype=np.float32)
    bias = np.asarray(b_attn, dtype=np.float32)
    # tri[p, f] = 1 where f >= p   (keep q >= k within the diagonal square)
    tri = (np.arange(P)[None, :] >= np.arange(P)[:, None]).astype(NPBF16)

    in_maps = []
    for c in range(NC):
        h0 = HPC * c
        cols = slice(h0 * HD, h0 * HD + HPC * HD)
        wqkv = np.concatenate(
            [W[:, cols], W[:, D:][:, cols], W[:, 2 * D:][:, cols]], axis=1)
        bq = np.concatenate(
            [bias[cols], bias[D:][cols], bias[2 * D:][cols]])
        in_maps.append({
            "xt": xt,
            "wqkv": np.ascontiguousarray(wqkv).astype(NPBF16),
            "bqkv": np.ascontiguousarray(bq),
            "wp": wp,
            "bp": bp,
            "tri": tri,
        })
    return in_maps


def kernel(**inputs):
    global LAST_RESULT
    nc = _build_module()
    in_maps = _make_in_maps(**inputs)
    trace = os.environ.get("KERNEL_TRACE", "0") == "1"
    res = bass_utils.run_bass_kernel_spmd(
        nc, in_maps, core_ids=list(range(NC)), trace=trace)
    LAST_RESULT = res
    y = np.empty((B, N, D), dtype=np.float32)
    for c in range(NC):
        yc = res.results[c]["out"]          # [512, 1024] = Y for token eighth c
        bidx, q = divmod(c, 4)
        y[bidx, q * TB:(q + 1) * TB, :] = np.asarray(yc, dtype=np.float32)
    return y


if __name__ == "__main__":
    # smoke test with random inputs
    rng = np.random.default_rng(0)
    inputs = {
        "hidden_states": rng.standard_normal((B, N, D), dtype=np.float32),
        "W_attn": (rng.standard_normal((D, 3 * D), dtype=np.float32) * D ** -0.5),
        "b_attn": rng.standard_normal((3 * D,), dtype=np.float32) * 0.02,
        "W_proj": (rng.standard_normal((D, D), dtype=np.float32) * D ** -0.5),
        "b_proj": rng.standard_normal((D,), dtype=np.float32) * 0.02,
    }
    y = kernel(**inputs)
    print("output", y.shape, y.dtype, float(np.abs(y).mean()))



# revision 5
# speedup vs baseline: 1.0942x; 1.0942x over previous
"""Distributed Bass kernel for nn_AttentionLayer_88545045774526.

Causal attention layer: B=2, N=2048, D=1024, H=16 heads of HD=64.
Sharding: tensor-parallel over heads -- each of the 8 cores owns 2 heads
(both batches, all tokens).  Attention outputs are redistributed
head-sharded -> token-sharded with ONE AllToAll PER BATCH (the batch-0
collective overlaps batch-1 attention compute), then each core runs the
output projection for its 2x256-token slices.

Matmul dtype: bf16 (fp32 streams at 1/4 PE rate); softmax in fp32.
Scores are computed transposed (S^T[k, q]) so that PV needs no on-chip
transpose of P; the softmax denominator rides a ones-column appended to V.
The softmax reciprocal runs on the scalar engine (ACT Reciprocal) -- a
[1,512] DVE reciprocal is lane-serial and costs 3.3us.
"""

import os
import sys

sys.path.insert(0, "/opt/trn_rl_repo")

import numpy as np
import ml_dtypes

import concourse.bass as bass
import concourse.mybir as mybir
import concourse.tile as tile
from concourse import bacc
from concourse import bass_utils
from concourse.masks import make_identity

BF16 = mybir.dt.bfloat16
F32 = mybir.dt.float32
NPBF16 = ml_dtypes.bfloat16

B, N, D = 2, 2048, 1024
H, HD = 16, 64
NC = 8                 # cores
HPC = 2                # heads per core
NT = B * N             # 4096 flat tokens (batch-major)
TB = 512               # token block (matmul moving dim)
SL = 256               # tokens per (core, batch) slice after AllToAll
P = 128

_BUILD_CACHE = {}
LAST_RESULT = None     # BassKernelResults of the most recent run (for test.py)


def _build_module(sim_mode=False, no_cc=False):
    """Build + compile the SPMD Bass graph (identical on all 8 cores)."""
    key = ("nc_sim" if sim_mode else "nc") + ("_nocc" if no_cc else "")
    if key in _BUILD_CACHE:
        return _BUILD_CACHE[key]

    nc = bacc.Bacc(
        "TRN2",
        target_bir_lowering=False,
        debug=False,
        enable_asserts=False,
        num_devices=1 if sim_mode else NC,
    )

    # ---- I/O ----
    xt = nc.dram_tensor("xt", [D, NT], BF16, kind="ExternalInput")        # X^T, replicated
    wqkv = nc.dram_tensor("wqkv", [D, 3 * P], BF16, kind="ExternalInput")  # [Q|K|V] cols, 2 heads
    bqkv = nc.dram_tensor("bqkv", [3 * P], F32, kind="ExternalInput")
    wp = nc.dram_tensor("wp", [D, D], BF16, kind="ExternalInput")          # full W_proj
    bp = nc.dram_tensor("bp", [D], F32, kind="ExternalInput")
    tri = nc.dram_tensor("tri", [P, P], BF16, kind="ExternalInput")        # tri[p,f]=1 if f>=p
    out = nc.dram_tensor("out", [2 * SL, D], F32, kind="ExternalOutput")   # Y rows: b*SL+t

    # collective bounce buffers (internal DRAM), one pair per batch
    cc_in = [nc.dram_tensor(f"cc_in{b}", [NC * P, SL], BF16, kind="Internal")
             for b in range(B)]
    cc_out = [nc.dram_tensor(f"cc_out{b}", [NC * P, SL], BF16, kind="Internal")
              for b in range(B)]

    NTB = NT // TB          # 8 token blocks
    DC = D // P             # 8 contraction chunks
    KBB = N // P            # 16 k-blocks per batch
    QB = N // TB            # 4 q-blocks per batch

    with tile.TileContext(nc) as tc:
        with (
            tc.tile_pool(name="consts", bufs=1) as consts,
            tc.tile_pool(name="xt_pool", bufs=1) as xt_pool,
            tc.tile_pool(name="pers", bufs=1) as pers,
            tc.tile_pool(name="mm_psum", bufs=2, space="PSUM") as mm_psum,
            tc.tile_pool(name="pss_psum", bufs=2, space="PSUM") as pss_psum,
            tc.tile_pool(name="pso_psum", bufs=1, space="PSUM") as pso_psum,
            tc.tile_pool(name="work", bufs=4) as work,
            tc.tile_pool(name="small", bufs=3) as small,
            tc.tile_pool(name="otb_pool", bufs=2) as otb_pool,
            tc.tile_pool(name="dramsc", bufs=3, space="DRAM") as dramsc,
        ):
            # ---- weights + first x blocks to SBUF, finely interleaved so
            # the first QKV matmuls can start ~2us after DMA start ----
            bq_sb = consts.tile([P, 3], F32)
            nc.sync.dma_start(bq_sb, bqkv[:].rearrange("(g p) -> p g", p=P))
            wqkv_sb = consts.tile([P, DC, 3 * P], BF16)
            xt_sb = xt_pool.tile([P, DC, NT], BF16)
            for dc in range(DC):
                nc.sync.dma_start(
                    wqkv_sb[:, dc],
                    wqkv[dc * P:(dc + 1) * P, :].rearrange("(c p) n -> p (c n)", p=P))
                for tb in range(2):
                    nc.sync.dma_start(
                        xt_sb[:, dc, tb * TB:(tb + 1) * TB],
                        xt[dc * P:(dc + 1) * P, tb * TB:(tb + 1) * TB].rearrange(
                            "(c p) n -> p (c n)", p=P))
            tri_sb = consts.tile([P, P], BF16)
            nc.sync.dma_start(tri_sb, tri[:, :])
            ident = consts.tile([P, P], BF16)
            make_identity(nc, ident)
            # preload the exp table set so the ~2.7us ACT_TABLE_LOAD hides here
            actwarm = consts.tile([1, 2], F32)
            nc.scalar.activation(actwarm[0:1, 0:1], bq_sb[0:1, 0:1],
                                 mybir.ActivationFunctionType.Exp)
            # PE warm-up: keep the PE busy while the first x/w DMAs land so
            # HAM un-throttles (4/8 -> 8/8) before the real matmuls start.
            warmsrc = consts.tile([P, TB], BF16)
            nc.vector.memset(warmsrc, 0.0)
            for i in range(12):
                pw = mm_psum.tile([P, TB], F32, tag="mm_ps", name="warm")
                nc.tensor.matmul(pw, lhsT=ident, rhs=warmsrc,
                                 start=True, stop=True)

            # ---- rest of X^T to SBUF ----
            for tb in range(2, NTB):
                nc.sync.dma_start(
                    xt_sb[:, :, tb * TB:(tb + 1) * TB],
                    xt[:, tb * TB:(tb + 1) * TB].rearrange("(c p) n -> p c n", p=P),
                )
            # needed only for the tail phases -- load after the hot-path DMAs
            bp_bc = consts.tile([P, D], F32)
            nc.sync.dma_start(bp_bc, bp[None, :].to_broadcast((P, D)))
            wp_sb = consts.tile([P, DC, D], BF16)
            nc.sync.dma_start(wp_sb, wp[:, :].rearrange("(c p) n -> p c n", p=P))

            # ---- QKV projection (transposed): qt/kt/vt = W^T X^T + b ----
            qt = pers.tile([P, NT], BF16)   # partitions: head h at rows h*64..h*64+64
            kt = pers.tile([P, NT], BF16)
            vt = pers.tile([P, NT], BF16)
            # vstore[:, kbg, h*65 : h*65+64] = V rows for token block kbg, head h
            # vstore[:, kbg, h*65+64] = 1.0  (softmax denominator trick)
            vstore = pers.tile([P, NT // P, 2 * (HD + 1)], BF16)
            nc.vector.memset(vstore[:, :, HD:HD + 1], 1.0)
            nc.vector.memset(vstore[:, :, 2 * HD + 1:2 * HD + 2], 1.0)
            dst = {0: qt, 1: kt, 2: vt}

            def emit_qkv_pair(tb0):
                # two token blocks at once: each stationary weight tile is
                # loaded once and streams both blocks back-to-back
                tbs = (tb0, tb0 + 1)
                for cg in range(3):
                    pss = [mm_psum.tile([P, TB], F32, tag="mm_ps",
                                        name=f"ps{i}") for i in range(2)]
                    for dc in range(DC):
                        for i, tb in enumerate(tbs):
                            nc.tensor.matmul(
                                pss[i],
                                lhsT=wqkv_sb[:, dc, cg * P:(cg + 1) * P],
                                rhs=xt_sb[:, dc, tb * TB:(tb + 1) * TB],
                                start=(dc == 0),
                                stop=(dc == DC - 1),
                            )
                    for i, tb in enumerate(tbs):
                        nc.vector.tensor_add(
                            out=dst[cg][:, tb * TB:(tb + 1) * TB], in0=pss[i],
                            in1=bq_sb[:, cg:cg + 1].to_broadcast((P, TB)),
                        )
                # V^T -> V natural (PE transpose) for these token blocks
                for kbg in range(4 * tb0, 4 * (tb0 + 2)):
                    pst = mm_psum.tile([P, P], BF16, tag="mm_ps", name="pst")
                    nc.tensor.transpose(pst, vt[:, kbg * P:(kbg + 1) * P], ident)
                    nc.vector.tensor_copy(out=vstore[:, kbg, 0:HD], in_=pst[:, 0:HD])
                    nc.vector.tensor_copy(
                        out=vstore[:, kbg, HD + 1:2 * HD + 1], in_=pst[:, HD:2 * HD])

            def emit_attn(b, qb):
                # S^T = (K^T block)^T-contraction @ Q^T, exp, PV
                qoff = b * N + qb * TB
                nkb = 4 * (qb + 1)          # causal: k blocks 0..4qb+3 (even)
                psO = [pso_psum.tile([HD + 1, TB], F32, tag=f"psO{h}",
                                     name=f"psO{h}")
                       for h in range(HPC)]
                for kp in range(nkb // 2):
                    pair = (2 * kp, 2 * kp + 1)
                    css = [max(kb - 4 * qb, 0) * P for kb in pair]
                    # two k-blocks share one 2-bank PSUM tile per head so a
                    # single exp covers both (halves ACT per-op overhead);
                    # S matmuls emitted h-alternating so row-group-disjoint
                    # (K=64) pairs can pipeline on the PE.
                    psS = [pss_psum.tile([P, 2 * TB], F32, tag="psS",
                                         name=f"psS{h}") for h in range(HPC)]
                    for i, kb in enumerate(pair):
                        koff = b * N + kb * P
                        for h in range(HPC):
                            hp = h * HD
                            nc.tensor.matmul(
                                psS[h][:, i * TB + css[i]:(i + 1) * TB],
                                lhsT=kt[hp:hp + HD, koff:koff + P],
                                rhs=qt[hp:hp + HD, qoff + css[i]:qoff + TB],
                                start=True, stop=True,
                            )
                    pts = []
                    for h in range(HPC):
                        pt = work.tile([P, 2 * TB], BF16, tag=f"pt{h}",
                                       name="pt")
                        nc.scalar.activation(
                            pt[:, css[0]:2 * TB], psS[h][:, css[0]:2 * TB],
                            mybir.ActivationFunctionType.Exp,
                            scale=float(HD) ** -0.5,
                        )
                        pts.append(pt)
                    for h in range(HPC):
                        for i, kb in enumerate(pair):
                            jj = kb - 4 * qb
                            if jj >= 0:
                                # triangular mask on the diagonal square
                                cc = i * TB + css[i]
                                nc.vector.tensor_mul(
                                    out=pts[h][:, cc:cc + P],
                                    in0=pts[h][:, cc:cc + P], in1=tri_sb)
                            nc.tensor.matmul(
                                psO[h][:, css[i]:TB],
                                lhsT=vstore[:, b * KBB + kb,
                                            h * (HD + 1):(h + 1) * (HD + 1)],
                                rhs=pts[h][:, i * TB + css[i]:(i + 1) * TB],
                                start=(kb == 0), stop=(kb == nkb - 1),
                            )
                # evacuate PSUM; the softmax reciprocal runs on DVE but in a
                # [64, 8] shape (a [1, 512] reciprocal is lane-serial, 3.3us;
                # reshaped via a DRAM bounce it is 0.18us), then the per-token
                # 1/den is broadcast across the 64 hd partitions via a second
                # DRAM bounce (DRAM source APs may broadcast).
                for h in range(HPC):
                    osb = small.tile([HD + 1, TB], F32, tag="osb", name="osb")
                    nc.vector.tensor_copy(out=osb, in_=psO[h])
                    d1 = dramsc.tile([TB], F32, tag="d1", name="d1")
                    nc.sync.dma_start(d1, osb[HD:HD + 1, :])
                    den64 = small.tile([HD, TB // HD], F32, tag="den64",
                                       name="den64")
                    nc.sync.dma_start(
                        den64, d1[:].rearrange("(p f) -> p f", p=HD))
                    rec64 = small.tile([HD, TB // HD], BF16, tag="rec64",
                                       name="rec64")
                    with nc.allow_low_precision(
                            reason="1/den in bf16: 0.4% rel on softmax scale "
                                   "is well inside the 2e-2 budget"):
                        nc.vector.reciprocal(rec64, den64)
                    d2 = dramsc.tile([TB], BF16, tag="d2", name="d2")
                    nc.sync.dma_start(
                        d2[:].rearrange("(p f) -> p f", p=HD), rec64)
                    recb = small.tile([HD, TB], BF16, tag="recb", name="recb")
                    nc.sync.dma_start(recb, d2[None, :].to_broadcast((HD, TB)))
                    ot = small.tile([HD, TB], BF16, tag="ot", name="ot")
                    nc.vector.tensor_mul(out=ot, in0=osb[0:HD, :], in1=recb)
                    for j in range(2):
                        s = 2 * qb + j
                        nc.sync.dma_start(
                            cc_in[b][s * P + h * HD: s * P + (h + 1) * HD, :],
                            ot[:, j * SL:(j + 1) * SL])

            def emit_a2a(b):
                if sim_mode or no_cc:
                    nc.sync.dma_start(cc_out[b][:, :], cc_in[b][:, :])
                else:
                    nc.gpsimd.collective_compute(
                        "AllToAll",
                        mybir.AluOpType.bypass,
                        replica_groups=[list(range(NC))],
                        ins=[cc_in[b][:, :]],
                        outs=[cc_out[b][:, :]],
                    )

            def emit_proj(b):
                # Y[b, my 256 tokens] = O^T^T W_p + bias, in two 128-token tiles
                for tk in range(SL // P):
                    otb = otb_pool.tile([P, DC, P], BF16, tag="otb", name="otb")
                    nc.sync.dma_start(
                        otb, cc_out[b][:, tk * P:(tk + 1) * P].rearrange(
                            "(c p) n -> p c n", p=P))
                    pss = [mm_psum.tile([P, TB], F32, tag="mm_ps",
                                        name=f"yp{i}") for i in range(2)]
                    for hc in range(DC):
                        for i in range(2):
                            nc.tensor.matmul(
                                pss[i],
                                lhsT=otb[:, hc, :],
                                rhs=wp_sb[:, hc, i * TB:(i + 1) * TB],
                                start=(hc == 0), stop=(hc == DC - 1),
                            )
                    ysb = otb_pool.tile([P, D], F32, tag="ysb", name="ysb")
                    for i in range(2):
                        nc.vector.tensor_add(
                            out=ysb[:, i * TB:(i + 1) * TB], in0=pss[i],
                            in1=bp_bc[:, i * TB:(i + 1) * TB],
                        )
                    nc.sync.dma_start(out[b * SL + tk * P: b * SL + (tk + 1) * P, :],
                                      ysb)

            # interleave: attn(b, qb) depends exactly on qkv blocks b*4..b*4+qb;
            # AllToAll for batch 0 is triggered halfway and overlaps batch-1
            # attention; the batch-0 projection covers the second AllToAll.
            for tp in range(NTB // 2):
                emit_qkv_pair(2 * tp)
                for t in (2 * tp, 2 * tp + 1):
                    emit_attn(t // 4, t % 4)
                if tp == 1:
                    emit_a2a(0)
            emit_a2a(1)
            emit_proj(0)
            emit_proj(1)

    nc.compile()
    _BUILD_CACHE[key] = nc
    return nc


def _make_in_maps(hidden_states, W_attn, b_attn, W_proj, b_proj):
    x = np.asarray(hidden_states, dtype=np.float32).reshape(NT, D)
    xt = np.ascontiguousarray(x.T).astype(NPBF16)
    wp = np.asarray(W_proj, dtype=np.float32).astype(NPBF16)
    bp = np.asarray(b_proj, dtype=np.float32)
    W = np.asarray(W_attn, dtype=np.float32)
    bias = np.asarray(b_attn, dtype=np.float32)
    # tri[p, f] = 1 where f >= p   (keep q >= k within the diagonal square)
    tri = (np.arange(P)[None, :] >= np.arange(P)[:, None]).astype(NPBF16)

    in_maps = []
    for c in range(NC):
        h0 = HPC * c
        cols = slice(h0 * HD, h0 * HD + HPC * HD)
        wqkv = np.concatenate(
            [W[:, cols], W[:, D:][:, cols], W[:, 2 * D:][:, cols]], axis=1)
        bq = np.concatenate(
            [bias[cols], bias[D:][cols], bias[2 * D:][cols]])
        in_maps.append({
            "xt": xt,
            "wqkv": np.ascontiguousarray(wqkv).astype(NPBF16),
            "bqkv": np.ascontiguousarray(bq),
            "wp": wp,
            "bp": bp,
            "tri": tri,
        })
    return in_maps


def kernel(**inputs):
    global LAST_RESULT
    nc = _build_module()
    in_maps = _make_in_maps(**inputs)
    trace = os.environ.get("KERNEL_TRACE", "0") == "1"
    res = bass_utils.run_bass_kernel_spmd(
        nc, in_maps, core_ids=list(range(NC)), trace=trace)
    LAST_RESULT = res
    y = np.empty((B, N, D), dtype=np.float32)
    for c in range(NC):
        yc = np.asarray(res.results[c]["out"], dtype=np.float32)  # [512, 1024]
        for b in range(B):
            y[b, c * SL:(c + 1) * SL, :] = yc[b * SL:(b + 1) * SL, :]
    return y


if __name__ == "__main__":
    # smoke test with random inputs
    rng = np.random.default_rng(0)
    inputs = {
        "hidden_states": rng.standard_normal((B, N, D), dtype=np.float32),
        "W_attn": (rng.standard_normal((D, 3 * D), dtype=np.float32) * D ** -0.5),
        "b_attn": rng.standard_normal((3 * D,), dtype=np.float32) * 0.02,
        "W_proj": (rng.standard_normal((D, D), dtype=np.float32) * D ** -0.5),
        "b_proj": rng.standard_normal((D,), dtype=np.float32) * 0.02,
    }
    y = kernel(**inputs)
    print("output", y.shape, y.dtype, float(np.abs(y).mean()))
